# revision 30
# baseline (speedup 1.0000x reference)
"""Causal GQA attention (S=2048, B=2, HQ=32, HKV=8, D=128) on 8 trn2 cores.

Sharding: the 16 (batch, kv-head) pairs are split 2 per core (data+head
parallel). Each pair carries group=4 query heads -> 8 attention heads/core.

Device kernel computes, per head, S^T = (Q K^T)^T in PSUM chunk-by-chunk
(so the softmax free axis never needs an on-chip transpose), exponentiates
on ACT into SBUF (P^T), applies the causal triangular mask only on the
128x128 diagonal block, then accumulates out^T = V^T-style matmuls with V
stationary and the softmax denominators with a ones-column matmul. All
matmul operands are viewed as float32r (full-rate fp32 on the PE array for
moving dim >= 256).

Host side only re-lays-out data: Q/K are fed pre-transposed [d, s], V as
[k_local, ktile, d], and the returned out^T [d, s] is transposed back.
"""

import numpy as np

import concourse.bass as bass
import concourse.mybir as mybir
import concourse.tile as tile
from concourse import bacc, bass_utils

S, B, HQ, HKV, D = 2048, 2, 32, 8, 128
G = HQ // HKV                      # 4 query heads per kv head
NCORES = 8
NPAIRS = B * HKV                   # 16 (batch, kv-head) pairs
PAIRS_PER_CORE = NPAIRS // NCORES  # 2
HEADS_PER_CORE = PAIRS_PER_CORE * G  # 8
SCALE = 1.0 / float(np.sqrt(D))
QC = 512                           # q-chunk (PSUM bank) width
NQC = S // QC                      # 4
KT = 128                           # k-tile (partition) width
NKT = S // KT                      # 16

F32 = mybir.dt.float32
F32R = mybir.dt.float32r
BF16 = mybir.dt.bfloat16


def emit_core_program(tc, qt, kt, v, recd, ot):
    """Emit the per-core program.

    qt: [HEADS_PER_CORE, D, S] f32r   Q^T per head ([d, q])
    kt: [PAIRS_PER_CORE, D, S] f32r   K^T per pair ([d, k])
    v:  [PAIRS_PER_CORE, 128, NKT*D] f32  V per pair ([k_local, kt, d])
    recd: [HEADS_PER_CORE, NQC, QC] f32 DRAM scratch for 1/sum rows
    ot: [HEADS_PER_CORE, D, S] f32   out^T per head ([d, q])

    QK^T runs in float32r (full-rate fp32); the P*V side runs in bf16
    (P in [0, e^~5], V order-1: bf16 keeps ~4e-3 relative accuracy and the
    softmax normalization cancels much of the P rounding).
    """
    from contextlib import ExitStack

    nc = tc.nc
    with ExitStack() as ctx:
        _emit_core_program(ctx, tc, nc, qt, kt, v, recd, ot)


def _emit_core_program(ctx, tc, nc, qt, kt, v, recd, ot):
    singles = ctx.enter_context(tc.tile_pool(name="singles", bufs=1))
    kv_pool = ctx.enter_context(tc.tile_pool(name="kv", bufs=2))
    q_pool = ctx.enter_context(tc.tile_pool(name="q", bufs=2))
    pt_pool = ctx.enter_context(tc.tile_pool(name="pt", bufs=2))
    ob_pool = ctx.enter_context(tc.tile_pool(name="ob", bufs=2))
    nrm_pool = ctx.enter_context(tc.tile_pool(name="nrm", bufs=2))
    ps_s = ctx.enter_context(tc.tile_pool(name="ps_s", bufs=1, space="PSUM"))
    ps_o = ctx.enter_context(tc.tile_pool(name="ps_o", bufs=1, space="PSUM"))
    ps_sum = ctx.enter_context(tc.tile_pool(name="ps_sum", bufs=1, space="PSUM"))

    # Constants
    # maskneg[k, q] = 0.0 where q >= k (allowed), -1e9 where q < k (masked).
    # Added to the S^T diagonal block before exp.
    maskneg = singles.tile([128, 128], F32)
    nc.gpsimd.memset(maskneg[:], 0.0)
    nc.gpsimd.affine_select(
        out=maskneg[:], in_=maskneg[:],
        compare_op=mybir.AluOpType.is_ge, fill=-1e9,
        base=0, pattern=[[1, 128]], channel_multiplier=-1,
    )
    onesc = singles.tile([128, 1], BF16)   # ones column (sum-over-k lhsT)
    nc.vector.memset(onesc[:], 1.0)

    for pair in range(PAIRS_PER_CORE):
        kt_sb = kv_pool.tile([D, S], F32R, tag="kt")
        nc.sync.dma_start(out=kt_sb[:], in_=kt[pair])
        v_sb = kv_pool.tile([128, NKT * D], BF16, tag="v")
        nc.gpsimd.dma_start(out=v_sb[:], in_=v[pair])  # casting DMA f32->bf16

        for g in range(G):
            head = pair * G + g
            q_sb = q_pool.tile([D, S], F32R)
            nc.sync.dma_start(out=q_sb[:], in_=qt[head])

            s_ps = ps_s.tile([128, 3 * QC], F32)    # 3 banks of S^T staging
            o_ps = ps_o.tile([128, S], F32)         # 4 banks: out^T accum
            sum_ps = ps_sum.tile([128, QC], F32)    # 1 bank: chunk c at row 32c
            p_sb = pt_pool.tile([128, S], BF16)     # P^T = exp(scale * S^T)



            for kti in range(NKT):
                w = KT * kti          # first allowed q for this k-tile
                c0 = w // QC          # first overlapping q-chunk

                def s_col(c, qcol):
                    # column inside s_ps for chunk c, in-chunk column qcol
                    return QC * ((c - c0) % 3) + qcol

                def qk(c):
                    off = max(0, w - QC * c)
                    nc.tensor.matmul(
                        out=s_ps[:, s_col(c, off):s_col(c, 0) + QC],
                        lhsT=kt_sb[:, w:w + KT],
                        rhs=q_sb[:, QC * c + off:QC * (c + 1)],
                        start=True, stop=True,
                    )

                def mask_diag():
                    # additive causal mask on the diagonal 128-wide block,
                    # which lives at in-chunk offset `off` of slot 0
                    off = w - QC * c0
                    nc.vector.tensor_add(
                        s_ps[:, off:off + KT], s_ps[:, off:off + KT],
                        maskneg[:])

                n_chunks = NQC - c0
                if n_chunks == 4:
                    for c in (0, 1, 2):
                        qk(c)
                    mask_diag()
                    # exp over chunks 0..2 (s_ps cols [w, 1536))
                    nc.scalar.activation(
                        p_sb[:, w:3 * QC], s_ps[:, w:3 * QC],
                        mybir.ActivationFunctionType.Exp, scale=SCALE)
                    qk(3)  # slot 0, reused after the exp above read it
                    nc.scalar.activation(
                        p_sb[:, 3 * QC:S], s_ps[:, 0:QC],
                        mybir.ActivationFunctionType.Exp, scale=SCALE)
                else:
                    for c in range(c0, NQC):
                        qk(c)
                    mask_diag()
                    base = w - QC * c0
                    nc.scalar.activation(
                        p_sb[:, w:S], s_ps[:, base:base + (S - w)],
                        mybir.ActivationFunctionType.Exp, scale=SCALE)

                for c in range(c0, NQC):
                    off = max(0, w - QC * c)
                    rhs = p_sb[:, QC * c + off:QC * (c + 1)]
                    first = kti == 0
                    last = kti == 4 * c + 3
                    nc.tensor.matmul(
                        out=o_ps[:, QC * c + off:QC * (c + 1)],
                        lhsT=v_sb[:, D * kti:D * (kti + 1)],
                        rhs=rhs, start=first, stop=last,
                    )
                    nc.tensor.matmul(
                        out=sum_ps[32 * c:32 * c + 1, off:QC],
                        lhsT=onesc[:],
                        rhs=rhs, start=first, stop=last,
                        tile_position=(0, 32 * c),
                    )

            # normalize: out^T[:, chunk c] *= 1 / sums[c] and store.
            # Broadcast 1/sum across partitions with a DRAM-bounce DMA
            # (partition-stride-0 read), keeping everything fp32.
            rec = nrm_pool.tile([128, QC], F32)
            bc_sb = nrm_pool.tile([128, 2 * QC], F32, tag="bc")
            osb = ob_pool.tile([128, S], F32)
            for c in range(NQC):
                row = slice(32 * c, 32 * c + 1)
                nc.vector.reciprocal(out=rec[row, :], in_=sum_ps[row, :])
                nc.sync.dma_start(out=recd[head, c], in_=rec[row, :])
                bcs = bc_sb[:, QC * (c % 2):QC * (c % 2) + QC]
                nc.gpsimd.dma_start(
                    out=bcs, in_=recd[head, c].partition_broadcast(128))
                nc.vector.tensor_mul(
                    osb[:, QC * c:QC * (c + 1)], o_ps[:, QC * c:QC * (c + 1)],
                    bcs)
                nc.sync.dma_start(
                    out=ot[head][:, QC * c:QC * (c + 1)],
                    in_=osb[:, QC * c:QC * (c + 1)])


_CACHED_NC = None


def build_program():
    global _CACHED_NC
    if _CACHED_NC is not None:
        return _CACHED_NC
    nc = bacc.Bacc("TRN2", target_bir_lowering=False, debug=False,
                   num_devices=NCORES)
    qt = nc.dram_tensor("qt", [HEADS_PER_CORE, D, S], F32R,
                        kind="ExternalInput").ap()
    kt = nc.dram_tensor("kt", [PAIRS_PER_CORE, D, S], F32R,
                        kind="ExternalInput").ap()
    v = nc.dram_tensor("v", [PAIRS_PER_CORE, 128, NKT * D], F32,
                       kind="ExternalInput").ap()
    recd = nc.dram_tensor("recd", [HEADS_PER_CORE, NQC, QC], F32,
                          kind="Internal").ap()
    ot = nc.dram_tensor("ot", [HEADS_PER_CORE, D, S], F32,
                        kind="ExternalOutput").ap()
    with tile.TileContext(nc) as tc:
        emit_core_program(tc, qt, kt, v, recd, ot)
    nc.compile()
    _CACHED_NC = nc
    return nc


def shard_inputs(query, key, value):
    """Full inputs -> list of 8 per-core in_maps (host-side relayout only)."""
    query = np.asarray(query, dtype=np.float32)
    key = np.asarray(key, dtype=np.float32)
    value = np.asarray(value, dtype=np.float32)

    # Q: [S,B,HQ,D] -> [B*HKV, G, D, S]
    qtall = np.ascontiguousarray(
        query.reshape(S, B, HKV, G, D).transpose(1, 2, 3, 4, 0)
    ).reshape(NPAIRS, G, D, S)
    # K: [S,B,HKV,D] -> [B*HKV, D, S]
    ktall = np.ascontiguousarray(
        key.transpose(1, 2, 3, 0)).reshape(NPAIRS, D, S)
    # V: [S,B,HKV,D] -> [B*HKV, k_local=128, NKT*D]
    vall = np.ascontiguousarray(
        value.reshape(NKT, 128, B, HKV, D).transpose(2, 3, 1, 0, 4)
    ).reshape(NPAIRS, 128, NKT * D)

    in_maps = []
    for c in range(NCORES):
        p0 = PAIRS_PER_CORE * c
        p1 = p0 + PAIRS_PER_CORE
        in_maps.append({
            "qt": np.ascontiguousarray(qtall[p0:p1].reshape(HEADS_PER_CORE, D, S)),
            "kt": np.ascontiguousarray(ktall[p0:p1]),
            "v": np.ascontiguousarray(vall[p0:p1]),
        })
    return in_maps


def unshard_output(results):
    """8 per-core {'ot': [8, D, S]} -> full [S, B, HQ, D]."""
    ot = np.stack([r["ot"] for r in results])          # [8, 8, D, S]
    ot = ot.reshape(B, HKV, G, D, S)                   # pairs major -> b, hkv
    out = np.ascontiguousarray(ot.transpose(4, 0, 1, 2, 3))  # [S,B,HKV,G,D]
    return out.reshape(S, B, HQ, D)


def kernel(query, key, value, _trace=False, _return_bkr=False):
    nc = build_program()
    in_maps = shard_inputs(query, key, value)
    bkr = bass_utils.run_bass_kernel_spmd(
        nc, in_maps, core_ids=list(range(NCORES)), trace=_trace)
    out = unshard_output(bkr.results)
    if _return_bkr:
        return out, bkr
    return out


if __name__ == "__main__":
    q = np.random.randn(S, B, HQ, D).astype(np.float32)
    k = np.random.randn(S, B, HKV, D).astype(np.float32)
    vv = np.random.randn(S, B, HKV, D).astype(np.float32)
    o = kernel(q, k, vv)
    print("out", o.shape, o.dtype, float(np.abs(o).max()))
